# revision 7
# baseline (speedup 1.0000x reference)
"""MoE feed-forward (top-1 routing, capacity 640, swiglu experts) on 8 trn2 cores.

Strategy (expert-parallel, as per sharding hint):
  * Host: router matmul/softmax/argmax + capacity-slot assignment (index
    plumbing, ~0.1% of FLOPs), gathers tokens per expert, shards 2 experts
    per core.
  * Device (Bass/Tile, per core): grouped GEMM  h = x @ W1  -> swiglu ->
    y = g @ W2, weighted by combine gates.  All matmuls in fp32r (full PE
    rate, fp32-class precision).  GEMM1 computes hT [feat, tok] so GEMM2
    needs no on-chip transpose.
  * Host: scatter weighted expert outputs back to token order; dense
    fallback FFN applied only to dropped tokens (none for typical loads).
"""

import os
import sys

import numpy as np


def _ensure_concourse():
    try:
        import concourse.bass  # noqa: F401
    except Exception:
        for p in ("/opt/trn_rl_repo", "/root/.axon_site/_ro/trn_rl_repo"):
            if os.path.isdir(p) and p not in sys.path:
                sys.path.insert(0, p)
        import concourse.bass  # noqa: F401


# Problem constants (hardcoded per the task contract).
B, S, D, H, E = 4, 2048, 768, 3072, 16
N = B * S
C = 640  # capacity per expert (ceil(1.25 * N / E))
FALLBACK_W = 1.0
NCORES = 8
EL = E // NCORES  # experts per core = 2
KD = D // 128  # 6 k-tiles for GEMM1 contraction
FB = (2 * H) // 128  # 48 feature blocks of GEMM1 output
FP = FB // 2  # 24 swiglu pairs == k-tiles of GEMM2 contraction
KH = H // 128  # 24
TOK = 320  # token half-tile (2 x 320 = 640); >=256 keeps fp32r at full rate
MT = C // 128  # 5 token m-tiles for GEMM2
DH = 384  # output d half-tile (2 x 384 = 768)

_NC = None  # cached Bass program
_WCACHE = {}  # weight reorder cache
LAST = None  # BassKernelResults of the most recent run (for profiling)


def _build_nc():
    """Per-core Bass program: 2 experts x (GEMM1 + swiglu + GEMM2 + gate)."""
    import concourse.bacc as bacc
    import concourse.bass as bass  # noqa: F401
    import concourse.mybir as mybir
    import concourse.tile as tile
    from contextlib import ExitStack

    f32 = mybir.dt.float32
    f32r = mybir.dt.float32r
    AF = mybir.ActivationFunctionType
    ALU = mybir.AluOpType

    nc = bacc.Bacc("TRN2", target_bir_lowering=False)
    # Host-side layouts are pre-tiled so every DMA is 2D [128, contiguous].
    xt = nc.dram_tensor("xt", [EL, 128, KD * C], f32r, kind="ExternalInput")
    w1r = nc.dram_tensor("w1r", [EL, FB, 128, KD * 128], f32r, kind="ExternalInput")
    w2t = nc.dram_tensor("w2t", [EL, 128, KH * D], f32r, kind="ExternalInput")
    b1t = nc.dram_tensor("b1t", [EL, 128, FB], f32, kind="ExternalInput")
    wce = nc.dram_tensor("wce", [EL, 128, MT], f32, kind="ExternalInput")
    y = nc.dram_tensor("y", [EL, C, D], f32, kind="ExternalOutput")

    with tile.TileContext(nc) as tc, ExitStack() as ctx:
        xp = ctx.enter_context(tc.tile_pool(name="xp", bufs=2))
        w2p = ctx.enter_context(tc.tile_pool(name="w2p", bufs=1))
        gp = ctx.enter_context(tc.tile_pool(name="gp", bufs=1))
        w1p = ctx.enter_context(tc.tile_pool(name="w1p", bufs=4))
        sap = ctx.enter_context(tc.tile_pool(name="sap", bufs=3))
        cst = ctx.enter_context(tc.tile_pool(name="cst", bufs=2))
        yp = ctx.enter_context(tc.tile_pool(name="yp", bufs=4))
        p1 = ctx.enter_context(tc.tile_pool(name="p1", bufs=3, space="PSUM"))
        p2 = ctx.enter_context(tc.tile_pool(name="p2", bufs=2, space="PSUM"))

        for e in range(EL):
            xsb = xp.tile([128, KD * C], f32r, tag="x")
            nc.gpsimd.dma_start(xsb[:], xt[e, :, :])
            w2sb = w2p.tile([128, KH * D], f32r, tag="w2")
            nc.gpsimd.dma_start(w2sb[:], w2t[e, :, :])
            b1sb = cst.tile([128, FB], f32, tag="b1")
            nc.gpsimd.dma_start(b1sb[:], b1t[e, :, :])
            wcsb = cst.tile([128, MT], f32, tag="wc")
            nc.gpsimd.dma_start(wcsb[:], wce[e, :, :])

            gt = gp.tile([128, KH * C], f32r, tag="g")

            # GEMM1 + swiglu: hT tiles [feat 128, tok 320]
            for fp in range(FP):
                w1a = w1p.tile([128, KD * 128], f32r, tag="w1a")
                nc.gpsimd.dma_start(w1a[:], w1r[e, fp, :, :])
                w1b = w1p.tile([128, KD * 128], f32r, tag="w1b")
                nc.gpsimd.dma_start(w1b[:], w1r[e, FP + fp, :, :])
                for t in range(2):
                    pa = p1.tile([128, TOK], f32, tag="pa")
                    pb = p1.tile([128, TOK], f32, tag="pb")
                    for k in range(KD):
                        nc.tensor.matmul(
                            pa[:],
                            lhsT=w1a[:, k * 128 : (k + 1) * 128],
                            rhs=xsb[:, k * C + t * TOK : k * C + (t + 1) * TOK],
                            start=(k == 0),
                            stop=(k == KD - 1),
                        )
                    for k in range(KD):
                        nc.tensor.matmul(
                            pb[:],
                            lhsT=w1b[:, k * 128 : (k + 1) * 128],
                            rhs=xsb[:, k * C + t * TOK : k * C + (t + 1) * TOK],
                            start=(k == 0),
                            stop=(k == KD - 1),
                        )
                    sa = sap.tile([128, TOK], f32, tag="sa")
                    # silu(a + b1_a)
                    nc.scalar.activation(
                        sa[:], pa[:], AF.Silu, bias=b1sb[:, fp : fp + 1], scale=1.0
                    )
                    # g = (b + b1_b) * silu(...)
                    nc.vector.scalar_tensor_tensor(
                        out=gt[:, fp * C + t * TOK : fp * C + (t + 1) * TOK],
                        in0=pb[:],
                        scalar=b1sb[:, FP + fp : FP + fp + 1],
                        in1=sa[:],
                        op0=ALU.add,
                        op1=ALU.mult,
                    )

            # GEMM2: y[tok 128, d 384] = sum_k g[tok, h_k] @ W2[h_k, d]
            for m in range(MT):
                for dh in range(2):
                    pt = p2.tile([128, DH], f32, tag="p2")
                    for k in range(KH):
                        nc.tensor.matmul(
                            pt[:],
                            lhsT=gt[:, k * C + m * 128 : k * C + m * 128 + 128],
                            rhs=w2sb[:, k * D + dh * DH : k * D + (dh + 1) * DH],
                            start=(k == 0),
                            stop=(k == KH - 1),
                        )
                    ysb = yp.tile([128, DH], f32, tag="y")
                    # weighted combine: y *= gate (per-token scalar); b2 is
                    # handled host-side (it is all zeros for this problem).
                    nc.scalar.activation(
                        ysb[:], pt[:], AF.Copy, bias=0.0, scale=wcsb[:, m : m + 1]
                    )
                    nc.gpsimd.dma_start(
                        y[e, m * 128 : (m + 1) * 128, dh * DH : (dh + 1) * DH], ysb[:]
                    )
    nc.compile()
    return nc


def _get_nc():
    global _NC
    if _NC is None:
        _NC = _build_nc()
    return _NC


def _reorder_weights(W1, W2, b1):
    key = (W1.__array_interface__["data"][0], W2.__array_interface__["data"][0])
    hit = _WCACHE.get(key)
    if hit is not None:
        return hit
    W1 = np.ascontiguousarray(W1, dtype=np.float32)
    W2 = np.ascontiguousarray(W2, dtype=np.float32)
    b1 = np.ascontiguousarray(b1, dtype=np.float32)
    # W1 [E, D, 2H] -> [E, FB, 128p(d within k), KD*128(f)]
    w1r = (
        W1.reshape(E, KD, 128, FB, 128)
        .transpose(0, 3, 2, 1, 4)
        .reshape(E, FB, 128, KD * 128)
        .copy()
    )
    # W2 [E, H, D] -> [E, 128p(h within k), KH*D]
    w2t = W2.reshape(E, KH, 128, D).transpose(0, 2, 1, 3).reshape(E, 128, KH * D).copy()
    # b1 [E, 2H] -> [E, 128, FB]
    b1t = b1.reshape(E, FB, 128).transpose(0, 2, 1).copy()
    out = (w1r, w2t, b1t)
    _WCACHE.clear()
    _WCACHE[key] = out
    return out


def _route(x_flat, Wr):
    logits = x_flat @ np.ascontiguousarray(Wr, dtype=np.float32)  # [N, E]
    lmax = logits.max(axis=-1, keepdims=True)
    p = np.exp(logits - lmax)
    gates = p / p.sum(axis=-1, keepdims=True)
    expert = np.argmax(gates, axis=-1)
    # slot = occurrence index of each token within its expert's queue
    order = np.argsort(expert, kind="stable")
    sorted_e = expert[order]
    starts = np.searchsorted(sorted_e, np.arange(E))
    within = np.arange(N) - starts[sorted_e]
    slot = np.empty(N, np.int64)
    slot[order] = within
    kept = slot < C
    top_idx = np.zeros((C, E), np.int32)
    valid = np.zeros((C, E), np.float32)
    tok = np.arange(N, dtype=np.int32)
    top_idx[slot[kept], expert[kept]] = tok[kept]
    valid[slot[kept], expert[kept]] = 1.0
    w_ce = gates[top_idx, np.arange(E)[None, :]].astype(np.float32) * valid  # [C, E]
    return gates, expert, kept, top_idx, valid, w_ce


def kernel(x, Wr, W1, b1, W2, b2, W1f, b1f, W2f, b2f, _trace=False):
    global LAST
    _ensure_concourse()
    from concourse.bass_utils import run_bass_kernel_spmd

    x_flat = np.ascontiguousarray(np.asarray(x).reshape(N, D), dtype=np.float32)
    gates, expert, kept, top_idx, valid, w_ce = _route(x_flat, np.asarray(Wr))
    w1r, w2t, b1t = _reorder_weights(np.asarray(W1), np.asarray(W2), np.asarray(b1))

    # Gather tokens per expert: [E, C, D]; invalid slots carry garbage rows,
    # they are zeroed by the gate weight (w_ce == 0 there).
    x_g = x_flat[top_idx.T]  # [E, C, D]
    # xT tiles: [E, 128p(d within k), KD*C]
    xT = x_g.reshape(E, C, KD, 128).transpose(0, 3, 2, 1).reshape(E, 128, KD * C)
    xT = np.ascontiguousarray(xT)
    # combine weights per expert: [E, 128, MT]
    wct = np.ascontiguousarray(w_ce.T.reshape(E, MT, 128).transpose(0, 2, 1))

    nc = _get_nc()
    in_maps = []
    for c in range(NCORES):
        sl = slice(c * EL, (c + 1) * EL)
        in_maps.append(
            {
                "xt": np.ascontiguousarray(xT[sl]),
                "w1r": np.ascontiguousarray(w1r[sl]),
                "w2t": np.ascontiguousarray(w2t[sl]),
                "b1t": np.ascontiguousarray(b1t[sl]),
                "wce": np.ascontiguousarray(wct[sl]),
            }
        )
    res = run_bass_kernel_spmd(nc, in_maps, list(range(NCORES)), trace=_trace)
    LAST = res

    # Combine: scatter weighted expert outputs back to token order.
    y_flat = np.zeros((N, D), np.float32)
    y_w = np.concatenate([r["y"] for r in res.results], axis=0)  # [E, C, D]
    mask = valid.astype(bool)  # [C, E]
    y_flat[top_idx[mask]] = y_w.transpose(1, 0, 2)[mask]
    if np.any(b2):
        eb = np.nonzero(mask)[1]
        y_flat[top_idx[mask]] += w_ce[mask][:, None] * np.asarray(b2)[eb]

    # Dense fallback for fully-dropped tokens (rare; none at typical loads).
    dropped = ~kept
    if np.any(dropped):
        xd = x_flat[dropped]
        hf = xd @ np.asarray(W1f) + np.asarray(b1f)
        gf = (hf[:, :H] / (1.0 + np.exp(-hf[:, :H]))) * hf[:, H:]
        y_flat[dropped] += FALLBACK_W * (gf @ np.asarray(W2f) + np.asarray(b2f))

    return y_flat.reshape(B, S, D)


# revision 9
# speedup vs baseline: 1.2150x; 1.2150x over previous
"""MoE feed-forward (top-1 routing, capacity 640, swiglu experts) on 8 trn2 cores.

Strategy (expert-parallel, as per sharding hint):
  * Host: router matmul/softmax/argmax + capacity-slot assignment (index
    plumbing, ~0.1% of FLOPs), gathers tokens per expert, shards 2 experts
    per core.
  * Device (Bass/Tile, per core): grouped GEMM  h = x @ W1  -> swiglu ->
    y = g @ W2, weighted by combine gates.  Matmuls in bf16 with fp32
    accumulate (bf16 keeps LDWEIGHTS off the critical path).  GEMM1
    computes hT [feat, tok] so GEMM2 needs no on-chip transpose.
  * Host: scatter weighted expert outputs back to token order; dense
    fallback FFN applied only to dropped tokens (none for typical loads).
"""

import os
import sys

import numpy as np


def _ensure_concourse():
    try:
        import concourse.bass  # noqa: F401
    except Exception:
        for p in ("/opt/trn_rl_repo", "/root/.axon_site/_ro/trn_rl_repo"):
            if os.path.isdir(p) and p not in sys.path:
                sys.path.insert(0, p)
        import concourse.bass  # noqa: F401


# Problem constants (hardcoded per the task contract).
B, S, D, H, E = 4, 2048, 768, 3072, 16
N = B * S
C = 640  # capacity per expert (ceil(1.25 * N / E))
FALLBACK_W = 1.0
NCORES = 8
EL = E // NCORES  # experts per core = 2
KD = D // 128  # 6 k-tiles for GEMM1 contraction
FB = (2 * H) // 128  # 48 feature blocks of GEMM1 output
FP = FB // 2  # 24 swiglu pairs == k-tiles of GEMM2 contraction
KH = H // 128  # 24
TOK = 320  # token half-tile (2 x 320 = 640); >=256 keeps fp32r at full rate
MT = C // 128  # 5 token m-tiles for GEMM2
DH = 384  # output d half-tile (2 x 384 = 768)

_NC = None  # cached Bass program
_WCACHE = {}  # weight reorder cache
LAST = None  # BassKernelResults of the most recent run (for profiling)


def _build_nc():
    """Per-core Bass program: 2 experts x (GEMM1 + swiglu + GEMM2 + gate)."""
    import concourse.bacc as bacc
    import concourse.bass as bass  # noqa: F401
    import concourse.mybir as mybir
    import concourse.tile as tile
    from contextlib import ExitStack

    f32 = mybir.dt.float32
    f32r = mybir.dt.float32r
    bf16 = mybir.dt.bfloat16
    AF = mybir.ActivationFunctionType
    ALU = mybir.AluOpType

    nc = bacc.Bacc("TRN2", target_bir_lowering=False)
    # Host-side layouts are pre-tiled so every DMA is 2D [128, contiguous].
    xt = nc.dram_tensor("xt", [EL, 128, KD * C], bf16, kind="ExternalInput")
    w1r = nc.dram_tensor("w1r", [EL, FB, 128, KD * 128], bf16, kind="ExternalInput")
    w2t = nc.dram_tensor("w2t", [EL, 128, KH * D], bf16, kind="ExternalInput")
    b1t = nc.dram_tensor("b1t", [EL, 128, FB], f32, kind="ExternalInput")
    wce = nc.dram_tensor("wce", [EL, 128, MT], f32, kind="ExternalInput")
    y = nc.dram_tensor("y", [EL, C, D], f32, kind="ExternalOutput")

    with tile.TileContext(nc) as tc, ExitStack() as ctx:
        xp = ctx.enter_context(tc.tile_pool(name="xp", bufs=2))
        w2p = ctx.enter_context(tc.tile_pool(name="w2p", bufs=2))
        gp = ctx.enter_context(tc.tile_pool(name="gp", bufs=2))
        w1p = ctx.enter_context(tc.tile_pool(name="w1p", bufs=4))
        sap = ctx.enter_context(tc.tile_pool(name="sap", bufs=3))
        cst = ctx.enter_context(tc.tile_pool(name="cst", bufs=2))
        yp = ctx.enter_context(tc.tile_pool(name="yp", bufs=4))
        p1 = ctx.enter_context(tc.tile_pool(name="p1", bufs=3, space="PSUM"))
        p2 = ctx.enter_context(tc.tile_pool(name="p2", bufs=2, space="PSUM"))

        for e in range(EL):
            xsb = xp.tile([128, KD * C], bf16, tag="x")
            nc.gpsimd.dma_start(xsb[:], xt[e, :, :])
            w2sb = w2p.tile([128, KH * D], bf16, tag="w2")
            nc.gpsimd.dma_start(w2sb[:], w2t[e, :, :])
            b1sb = cst.tile([128, FB], f32, tag="b1")
            nc.gpsimd.dma_start(b1sb[:], b1t[e, :, :])
            wcsb = cst.tile([128, MT], f32, tag="wc")
            nc.gpsimd.dma_start(wcsb[:], wce[e, :, :])

            gt = gp.tile([128, KH * C], bf16, tag="g")

            # GEMM1 + swiglu: hT tiles [feat 128, tok 320]
            for fp in range(FP):
                w1a = w1p.tile([128, KD * 128], bf16, tag="w1a")
                nc.gpsimd.dma_start(w1a[:], w1r[e, fp, :, :])
                w1b = w1p.tile([128, KD * 128], bf16, tag="w1b")
                nc.gpsimd.dma_start(w1b[:], w1r[e, FP + fp, :, :])
                for t in range(2):
                    pa = p1.tile([128, TOK], f32, tag="pa")
                    pb = p1.tile([128, TOK], f32, tag="pb")
                    for k in range(KD):
                        nc.tensor.matmul(
                            pa[:],
                            lhsT=w1a[:, k * 128 : (k + 1) * 128],
                            rhs=xsb[:, k * C + t * TOK : k * C + (t + 1) * TOK],
                            start=(k == 0),
                            stop=(k == KD - 1),
                        )
                    for k in range(KD):
                        nc.tensor.matmul(
                            pb[:],
                            lhsT=w1b[:, k * 128 : (k + 1) * 128],
                            rhs=xsb[:, k * C + t * TOK : k * C + (t + 1) * TOK],
                            start=(k == 0),
                            stop=(k == KD - 1),
                        )
                    sa = sap.tile([128, TOK], f32, tag="sa")
                    # silu(a + b1_a)
                    nc.scalar.activation(
                        sa[:], pa[:], AF.Silu, bias=b1sb[:, fp : fp + 1], scale=1.0
                    )
                    # g = (b + b1_b) * silu(...)
                    nc.vector.scalar_tensor_tensor(
                        out=gt[:, fp * C + t * TOK : fp * C + (t + 1) * TOK],
                        in0=pb[:],
                        scalar=b1sb[:, FP + fp : FP + fp + 1],
                        in1=sa[:],
                        op0=ALU.add,
                        op1=ALU.mult,
                    )

            # GEMM2: y[tok 128, d 384] = sum_k g[tok, h_k] @ W2[h_k, d]
            for m in range(MT):
                for dh in range(2):
                    pt = p2.tile([128, DH], f32, tag="p2")
                    for k in range(KH):
                        nc.tensor.matmul(
                            pt[:],
                            lhsT=gt[:, k * C + m * 128 : k * C + m * 128 + 128],
                            rhs=w2sb[:, k * D + dh * DH : k * D + (dh + 1) * DH],
                            start=(k == 0),
                            stop=(k == KH - 1),
                        )
                    ysb = yp.tile([128, DH], f32, tag="y")
                    # weighted combine: y *= gate (per-token scalar); b2 is
                    # handled host-side (it is all zeros for this problem).
                    nc.scalar.activation(
                        ysb[:], pt[:], AF.Copy, bias=0.0, scale=wcsb[:, m : m + 1]
                    )
                    nc.gpsimd.dma_start(
                        y[e, m * 128 : (m + 1) * 128, dh * DH : (dh + 1) * DH], ysb[:]
                    )
    nc.compile()
    return nc


def _get_nc():
    global _NC
    if _NC is None:
        _NC = _build_nc()
    return _NC


def _reorder_weights(W1, W2, b1):
    key = (W1.__array_interface__["data"][0], W2.__array_interface__["data"][0])
    hit = _WCACHE.get(key)
    if hit is not None:
        return hit
    W1 = np.ascontiguousarray(W1, dtype=np.float32)
    W2 = np.ascontiguousarray(W2, dtype=np.float32)
    b1 = np.ascontiguousarray(b1, dtype=np.float32)
    # W1 [E, D, 2H] -> [E, FB, 128p(d within k), KD*128(f)]
    import ml_dtypes

    w1r = np.ascontiguousarray(
        W1.reshape(E, KD, 128, FB, 128)
        .transpose(0, 3, 2, 1, 4)
        .reshape(E, FB, 128, KD * 128)
        .astype(ml_dtypes.bfloat16)
    )
    # W2 [E, H, D] -> [E, 128p(h within k), KH*D]
    w2t = np.ascontiguousarray(
        W2.reshape(E, KH, 128, D)
        .transpose(0, 2, 1, 3)
        .reshape(E, 128, KH * D)
        .astype(ml_dtypes.bfloat16)
    )
    # b1 [E, 2H] -> [E, 128, FB]
    b1t = b1.reshape(E, FB, 128).transpose(0, 2, 1).copy()
    out = (w1r, w2t, b1t)
    _WCACHE.clear()
    _WCACHE[key] = out
    return out


def _route(x_flat, Wr):
    logits = x_flat @ np.ascontiguousarray(Wr, dtype=np.float32)  # [N, E]
    lmax = logits.max(axis=-1, keepdims=True)
    p = np.exp(logits - lmax)
    gates = p / p.sum(axis=-1, keepdims=True)
    expert = np.argmax(gates, axis=-1)
    # slot = occurrence index of each token within its expert's queue
    order = np.argsort(expert, kind="stable")
    sorted_e = expert[order]
    starts = np.searchsorted(sorted_e, np.arange(E))
    within = np.arange(N) - starts[sorted_e]
    slot = np.empty(N, np.int64)
    slot[order] = within
    kept = slot < C
    top_idx = np.zeros((C, E), np.int32)
    valid = np.zeros((C, E), np.float32)
    tok = np.arange(N, dtype=np.int32)
    top_idx[slot[kept], expert[kept]] = tok[kept]
    valid[slot[kept], expert[kept]] = 1.0
    w_ce = gates[top_idx, np.arange(E)[None, :]].astype(np.float32) * valid  # [C, E]
    return gates, expert, kept, top_idx, valid, w_ce


def kernel(x, Wr, W1, b1, W2, b2, W1f, b1f, W2f, b2f, _trace=False):
    global LAST
    _ensure_concourse()
    from concourse.bass_utils import run_bass_kernel_spmd

    x_flat = np.ascontiguousarray(np.asarray(x).reshape(N, D), dtype=np.float32)
    gates, expert, kept, top_idx, valid, w_ce = _route(x_flat, np.asarray(Wr))
    w1r, w2t, b1t = _reorder_weights(np.asarray(W1), np.asarray(W2), np.asarray(b1))

    # Gather tokens per expert: [E, C, D]; invalid slots carry garbage rows,
    # they are zeroed by the gate weight (w_ce == 0 there).
    x_g = x_flat[top_idx.T]  # [E, C, D]
    # xT tiles: [E, 128p(d within k), KD*C]
    import ml_dtypes

    xT = x_g.reshape(E, C, KD, 128).transpose(0, 3, 2, 1).reshape(E, 128, KD * C)
    xT = np.ascontiguousarray(xT, dtype=ml_dtypes.bfloat16)
    # combine weights per expert: [E, 128, MT]
    wct = np.ascontiguousarray(w_ce.T.reshape(E, MT, 128).transpose(0, 2, 1))

    nc = _get_nc()
    in_maps = []
    for c in range(NCORES):
        sl = slice(c * EL, (c + 1) * EL)
        in_maps.append(
            {
                "xt": np.ascontiguousarray(xT[sl]),
                "w1r": np.ascontiguousarray(w1r[sl]),
                "w2t": np.ascontiguousarray(w2t[sl]),
                "b1t": np.ascontiguousarray(b1t[sl]),
                "wce": np.ascontiguousarray(wct[sl]),
            }
        )
    res = run_bass_kernel_spmd(nc, in_maps, list(range(NCORES)), trace=_trace)
    LAST = res

    # Combine: scatter weighted expert outputs back to token order.
    y_flat = np.zeros((N, D), np.float32)
    y_w = np.concatenate([r["y"] for r in res.results], axis=0)  # [E, C, D]
    mask = valid.astype(bool)  # [C, E]
    y_flat[top_idx[mask]] = y_w.transpose(1, 0, 2)[mask]
    if np.any(b2):
        eb = np.nonzero(mask)[1]
        y_flat[top_idx[mask]] += w_ce[mask][:, None] * np.asarray(b2)[eb]

    # Dense fallback for fully-dropped tokens (rare; none at typical loads).
    dropped = ~kept
    if np.any(dropped):
        xd = x_flat[dropped]
        hf = xd @ np.asarray(W1f) + np.asarray(b1f)
        gf = (hf[:, :H] / (1.0 + np.exp(-hf[:, :H]))) * hf[:, H:]
        y_flat[dropped] += FALLBACK_W * (gf @ np.asarray(W2f) + np.asarray(b2f))

    return y_flat.reshape(B, S, D)
